# revision 28
# baseline (speedup 1.0000x reference)
"""Trainium2 Bass kernel for GBMS mean-shift step (nn_GBMS_RNN_137438953906).

Math (per batch b):
    W = exp((X X^T - 1) / bandwidth^2)          [N, N]
    Y = (W @ X) / rowsum(W)                     [N, D]
    out = Y / max(||Y||_2, 1e-12)  (L2 norm along D)

rowsum(W) is a positive per-row scalar, so it cancels in the final L2
normalization; we never compute row sums.  Uniform scales on X cancel the
same way, so X is carried as 8*X (fp8-friendly range, exact power of 2).

Sharding: data-parallel over batch B=8 across the 8 NeuronCores.

Per-core dataflow (N=4096 as 8 column stripes of 512; W tiles are
[128 j-rows x 512 stripe-cols], 32 j-blocks per stripe):
  xt8[d64, 2, n] = 8*X^T in fp8e4m3  (PE half-transposes of bf16 8X + DVE
      convert; the [64,2] split is the DoubleRow matmul's paired-K layout)
  direct tile (jb, g):  S = xt8_jb^T xt8_g   (fp8 DoubleRow, 0.5 cyc/row)
                        W = exp(S/(64 b^2) - 1/b^2) -> bf16
                        (ACT, 2-tile batches, runtime scale/bias APs)
  symmetry reuse: W is symmetric, so the 4g tiles of stripe g above the
      diagonal are never recomputed: when stripe g' finishes the 4-tile
      group destined for stripe gd, ONE wide XBAR DMA-transpose turns the
      group [128, 2048] into wr [128, 16, 128] whose strided views
      wr[:, q::4, :] are ready-to-use transposed rhs tiles for stripe gd.
      This removes 44% of the exp work (ACT is the co-bottleneck with PE)
      and 44% of the S matmuls, at zero PE/ACT cost (DMA+HWDGE are idle).
  Y accumulation: yt[d, n512] += x16_jb^T @ W_tile  (bf16 matmuls, PSUM).
  Tail per stripe: yt -> bf16 stage -> PE transpose -> y16[n, d]; squares
      + row-reduce on DVE; fast-inverse-sqrt normalization (DVE bit trick
      + 1 Newton step, ~0.2% worst case); f32 stores, spread across the
      back stripes so the DVE/store burst overlaps PE work.

The emission schedule software-pipelines across stripes: each stripe's
S/exp stream runs one batch ahead of its Y-matmul stream, and a stripe's
leftover Y matmuls drain a few per cycle inside the next stripe so ACT
never waits behind a matmul burst at stripe boundaries.

fp8/bf16 error budget: measured 2.5e-3 overall at b=0.1 (output-side bf16
rounding) and 1.25e-2 at b=1.0 (fp8 X quantization common-mode term),
both under the 2e-2 gate.
"""

import sys

if "/opt/trn_rl_repo" not in sys.path:
    sys.path.insert(0, "/opt/trn_rl_repo")

import numpy as np

import concourse.mybir as mybir
from concourse import bacc
from concourse.tile import TileContext
from concourse.bass_utils import run_bass_kernel_spmd
from concourse.masks import make_identity

P = 128
N = 4096
D = 128
NB = N // P  # 32 row blocks
G = N // 512  # 8 column stripes
NCHUNK = 8  # input DMA chunks (4 row-blocks each)

F32 = mybir.dt.float32
BF16 = mybir.dt.bfloat16
FP8 = mybir.dt.float8e4
I32 = mybir.dt.int32
DR = mybir.MatmulPerfMode.DoubleRow

_CACHED_NC = None


def _build():
    nc = bacc.Bacc("TRN2", target_bir_lowering=False, debug=False)

    x_in = nc.dram_tensor("X", [N, D], F32, kind="ExternalInput")
    bw_in = nc.dram_tensor("bandwidth", [1], F32, kind="ExternalInput")
    y_out = nc.dram_tensor("Y", [N, D], F32, kind="ExternalOutput")

    x_src = x_in.rearrange("(jb p) d -> p jb d", p=P)  # [128, 32, 128] view
    y_dst = y_out.rearrange("(nb p) d -> p nb d", p=P)

    with TileContext(nc) as tc:
        with (
            tc.tile_pool(name="const", bufs=1) as const,
            tc.tile_pool(name="bigf32", bufs=1) as bigf32,
            tc.tile_pool(name="svpool", bufs=2) as sv_pool,
            tc.tile_pool(name="wrpool", bufs=17) as wr_pool,
            tc.tile_pool(name="sqpool", bufs=2) as sq_pool,
            tc.tile_pool(name="stgpool", bufs=2) as stg_pool,
            tc.tile_pool(name="spsum", bufs=2, space="PSUM") as s_pool,
            tc.tile_pool(name="ytpsum", bufs=2, space="PSUM") as yt_pool,
            tc.tile_pool(name="tppsum", bufs=2, space="PSUM") as tp_pool,
        ):
            # ---- input DMAs: chunk 0 first (it gates the pipeline) ----
            x_nat = bigf32.tile([P, NB, D], F32, tag="big", name="x_nat")
            cb = NB // NCHUNK  # 4 row blocks per chunk
            nc.sync.dma_start(x_nat[:, 0:cb, :], x_src[:, 0:cb, :])

            # bf16 identity first on the Pool queue so the PE warm-up can
            # start before the bandwidth SWDGE transfer completes
            identb = const.tile([P, P], BF16)
            make_identity(nc, identb[:])

            bw = const.tile([P, 1], F32)
            nc.gpsimd.dma_start(bw[:], bw_in[None, :].to_broadcast([P, 1]))

            for c in range(1, NCHUNK):
                nc.sync.dma_start(
                    x_nat[:, c * cb : (c + 1) * cb, :],
                    x_src[:, c * cb : (c + 1) * cb, :],
                )

            # ---- runtime scalars ----
            scr = const.tile([P, 5], F32)
            bsq = scr[:, 0:1]
            rb2 = scr[:, 1:2]
            negc = scr[:, 2:3]
            sc64 = scr[:, 3:4]
            dummy = scr[:, 4:5]
            # preload the Exp ACT table immediately (memset scratch input so
            # the 1.3us table load never waits for the bandwidth transfer)
            nc.vector.memset(dummy, 1.0)
            nc.scalar.activation(dummy, dummy, mybir.ActivationFunctionType.Exp)
            nc.vector.tensor_tensor(bsq, bw[:], bw[:], mybir.AluOpType.mult)
            nc.vector.reciprocal(rb2, bsq)  # 1/b^2
            nc.vector.tensor_scalar_mul(negc, rb2, -1.0)  # -1/b^2
            nc.vector.tensor_scalar_mul(sc64, rb2, 1.0 / 64.0)  # 1/(64 b^2)

            x16 = const.tile([P, NB, D], BF16)  # 8*X, Y-matmul lhsT
            xt8 = const.tile([64, 2, N], FP8)  # 8*X^T, S-matmul operands

            # PE warm-up junk transposes: keep PE continuously busy from
            # identity-ready until chunk 0's real work so the clock-ramp
            # timer (full speed after 3us of uninterrupted busy) never
            # resets during the DMA wait
            warm = s_pool.tile([P, 2, 512], F32, tag="s", name="warm")
            warmb = warm.bitcast(BF16)
            for t in range(18):
                nc.tensor.transpose(
                    warmb[:, t % 2, (t % 4) * P : (t % 4 + 1) * P],
                    identb[:],
                    identb[:],
                )

            chunks_done = [0]

            def emit_chunk(c):
                blk = slice(c * cb, (c + 1) * cb)
                nc.vector.tensor_scalar_mul(x16[:, blk, :], x_nat[:, blk, :], 8.0)
                xtp = tp_pool.tile([64, 2, 512], BF16, tag="tp", name="xtp")
                for o in range(cb):
                    jb = c * cb + o
                    for i in range(2):
                        nc.tensor.transpose(
                            xtp[:, i, o * P : (o + 1) * P],
                            x16[:, jb, i * 64 : (i + 1) * 64],
                            identb[:],
                        )
                nc.vector.tensor_copy(xt8[:, :, c * 512 : (c + 1) * 512], xtp[:])

            def need_chunks(upto):
                while chunks_done[0] <= min(upto, NCHUNK - 1):
                    emit_chunk(chunks_done[0])
                    chunks_done[0] += 1

            # ---- output staging ----
            y16 = const.tile([P, NB, D], BF16)  # [n_in_block, nb, d]
            ss_all = const.tile([P, NB], F32)
            tmp = const.tile([P, NB], F32)
            rcp = const.tile([P, NB], F32)
            magic = const.tile([P, NB], I32)
            shreg = const.tile([P, NB], I32)
            nc.vector.memset(magic[:], 0x5F3759DF)
            y_stage_box = [None]

            def normalize_and_store(g0, g1):
                """L2-normalize output rows of stripes [g0, g1) and DMA out.
                1/norm via the fast-inverse-sqrt bit trick + ONE Newton step
                (max rel err ~0.2%, well inside the error budget).  ss is
                always well away from 0: W's diagonal is ~1 so |Y| >= ~8."""
                y_stage = y_stage_box[0]
                lo, hi = g0 * 4, g1 * 4  # nb range
                ss = ss_all[:, lo:hi]
                rs = rcp[:, lo:hi]
                tm = tmp[:, lo:hi]
                nc.vector.tensor_scalar(
                    shreg[:, lo:hi],
                    ss.bitcast(I32),
                    1,
                    None,
                    mybir.AluOpType.logical_shift_right,
                )
                nc.vector.tensor_tensor(
                    rs.bitcast(I32),
                    magic[:, lo:hi],
                    shreg[:, lo:hi],
                    mybir.AluOpType.subtract,
                )
                nc.vector.tensor_tensor(tm, rs, rs, mybir.AluOpType.mult)
                nc.vector.tensor_tensor(tm, tm, ss, mybir.AluOpType.mult)
                nc.vector.tensor_scalar(
                    tm, tm, -0.5, 1.5, mybir.AluOpType.mult, mybir.AluOpType.add
                )
                nc.vector.tensor_tensor(rs, rs, tm, mybir.AluOpType.mult)
                for nb in range(lo, hi):
                    nc.vector.tensor_scalar_mul(
                        y_stage[:, nb, :], y16[:, nb, :], rcp[:, nb : nb + 1]
                    )
                mid = (lo + hi) // 2
                nc.sync.dma_start(y_dst[:, lo:mid, :], y_stage[:, lo:mid, :])
                nc.sync.dma_start(y_dst[:, mid:hi, :], y_stage[:, mid:hi, :])

            def make_tail(g, stg, fine=False):
                """Tail of stripe g: stg (= yt in bf16) -> y16[n, d] via one
                wide XBAR DMA-transpose (PE-free), then per-block fused
                square+reduce (DVE tensor_tensor_reduce).  fine mode (last
                stripe) uses PE transposes instead: lower latency since the
                DMA path costs ~1.7us end-to-end."""

                def tail():
                    nbs0 = g * 4
                    if fine:
                        for h in range(2):
                            tp = tp_pool.tile([P, 4, P], BF16, tag="tp", name="tp")
                            for t in range(2):
                                tt = h * 2 + t
                                nc.tensor.transpose(
                                    tp[:, t, :],
                                    stg[:, tt * P : (tt + 1) * P],
                                    identb[:],
                                )
                            nc.vector.tensor_copy(
                                y16[:, nbs0 + h * 2 : nbs0 + h * 2 + 2, :],
                                tp[:, 0:2, :],
                            )
                            sqt = sq_pool.tile([P, 2, P], F32, tag="sq", name="sqt")
                            nbs = slice(nbs0 + h * 2, nbs0 + h * 2 + 2)
                            nc.vector.tensor_tensor(
                                sqt[:], y16[:, nbs, :], y16[:, nbs, :],
                                mybir.AluOpType.mult,
                            )
                            nc.vector.tensor_reduce(
                                ss_all[:, nbs],
                                sqt[:],
                                axis=mybir.AxisListType.X,
                                op=mybir.AluOpType.add,
                            )
                    else:
                        nc.sync.dma_start_transpose(
                            y16[:, nbs0 : nbs0 + 4, :], stg[:]
                        )
                        sqt = sq_pool.tile([P, 4, P], F32, tag="sq", name="sqt")
                        nbs = slice(nbs0, nbs0 + 4)
                        nc.gpsimd.tensor_tensor(
                            sqt[:], y16[:, nbs, :], y16[:, nbs, :],
                            mybir.AluOpType.mult,
                        )
                        nc.vector.tensor_reduce(
                            ss_all[:, nbs],
                            sqt[:],
                            axis=mybir.AxisListType.X,
                            op=mybir.AluOpType.add,
                        )

                return tail

            wr_tiles = {}  # (gs, gd) -> wide-transposed 4-tile group
            # cross-stripe pipeline state: leftover Y matmuls of the previous
            # stripe drain a few per cycle so the next stripe's S/exp are
            # never stuck behind a matmul burst; then its stg copy + tail.
            state = {"carry": [], "stg": None, "tail": None}
            CR = 12  # carry drain rate per cycle

            # per-stripe Y-accumulation state persists across the loop so a
            # stripe's reuse-Y matmuls can be pre-released during the
            # previous (ACT-bound) stripe
            sstates = [{"yt": None, "n": 0, "pre": 0} for _ in range(G)]

            def emit_y_for(gy, jb, rhs):
                st = sstates[gy]
                if st["yt"] is None:
                    st["yt"] = yt_pool.tile([P, 512], F32, tag="yt", name="yt")
                nc.tensor.matmul(
                    st["yt"][:],
                    x16[:, jb, :],
                    rhs,
                    start=(st["n"] == 0),
                    stop=(st["n"] == 31),
                )
                st["n"] += 1

            # ---- main loop over column stripes ----
            for g in range(G):
                ndirect = 32 - 4 * g
                batches = [[s, s + 1] for s in range(0, ndirect, 2)]

                sv = sv_pool.tile([P, 32, 512], BF16, tag="sv", name="sv")

                def emit_y(jb, rhs, g=g):
                    emit_y_for(g, jb, rhs)

                rq = []
                for jb in range(sstates[g]["pre"], 4 * g):
                    gs, q = jb // 4, jb % 4
                    rq.append((jb, wr_tiles[(gs, g)][:, q:16:4, :]))
                rpc = -(-len(rq) // len(batches))  # ceil: spread over cycles

                dq_ready = []  # direct (jb, rhs) whose exp has been emitted

                def emit_batch(slots):
                    # S matmuls (fp8 DoubleRow) + exp batch -> sv (bf16)
                    if g == 0:
                        need_chunks(min(slots[-1] // cb + 1, NCHUNK - 1))
                    s_t = s_pool.tile([P, 2, 512], F32, tag="s", name="s_t")
                    for q, sl in enumerate(slots):
                        jb = 4 * g + sl
                        nc.tensor.matmul(
                            s_t[:, q, :],
                            xt8[:, :, jb * P : (jb + 1) * P],
                            xt8[:, :, g * 512 : (g + 1) * 512],
                            start=True,
                            stop=True,
                            perf_mode=DR,
                        )
                    nc.scalar.activation(
                        sv[:, slots[0] : slots[-1] + 1, :],
                        s_t[:],
                        mybir.ActivationFunctionType.Exp,
                        bias=negc,
                        scale=sc64,
                    )
                    for sl in slots:
                        dq_ready.append((4 * g + sl, sv[:, sl, :]))
                        if sl % 4 == 3 and sl >= 4:
                            gd = g + sl // 4
                            wr = wr_pool.tile(
                                [P, 16, P], BF16, tag="wr", name="wr"
                            )
                            wr_tiles[(g, gd)] = wr
                            nc.sync.dma_start_transpose(
                                wr[:],
                                sv[:, sl - 3 : sl + 1, :].rearrange(
                                    "p a b -> p (a b)"
                                ),
                            )

                for k in range(len(batches)):
                    # S/exp run one batch ahead of the Y stream
                    if k == 0:
                        emit_batch(batches[0])
                        if len(batches) > 1:
                            emit_batch(batches[1])
                    elif k + 1 < len(batches):
                        emit_batch(batches[k + 1])
                    slots = batches[k]
                    # drain the previous stripe's leftovers, then its stg
                    # copy + tail
                    for _ in range(CR):
                        if state["carry"]:
                            state["carry"].pop(0)()
                    if not state["carry"] and state["stg"] is not None:
                        state["stg"]()
                        state["stg"] = None
                        state["tail"]()
                        state["tail"] = None
                    # reuse-Y fillers (no ACT dependency)
                    for _ in range(rpc):
                        if rq:
                            emit_y(*rq.pop(0))
                    # direct-Y, trailing the lookahead exp batches deeply
                    # (extra slack hides the exp->matmul semaphore joins)
                    while len(dq_ready) > 3 * len(slots):
                        emit_y(*dq_ready.pop(0))
                    # normalization spread over the back stripes
                    if g == G - 2 and k == 3:
                        y_stage_box[0] = bigf32.tile(
                            [P, NB, D], F32, tag="big", name="y_stage"
                        )
                        normalize_and_store(0, 3)
                    if g == G - 1 and k == 0:
                        normalize_and_store(3, 5)
                    if g == G - 1 and k == 1:
                        normalize_and_store(5, G - 1)

                def make_carry(e, emit_y=emit_y):
                    return lambda: emit_y(*e)

                state["carry"] = [make_carry(e) for e in rq + dq_ready]

                def make_stg(g=g):
                    def stg_fn():
                        assert sstates[g]["n"] == 32, (g, sstates[g]["n"])
                        stg = stg_pool.tile(
                            [P, 512], BF16, tag="stg", name="stg"
                        )
                        nc.vector.tensor_copy(stg[:], sstates[g]["yt"][:])
                        state["tail"] = make_tail(g, stg, fine=True)

                    return stg_fn

                state["stg"] = make_stg()

            while state["carry"]:
                state["carry"].pop(0)()
            state["stg"]()
            state["tail"]()
            normalize_and_store(G - 1, G)

    nc.compile()
    return nc


def _get_nc():
    global _CACHED_NC
    if _CACHED_NC is None:
        _CACHED_NC = _build()
    return _CACHED_NC


def kernel(X: np.ndarray, bandwidth: np.ndarray, **run_kwargs):
    """Full-input entry point: X [8, 4096, 128] f32, bandwidth scalar f32.

    Returns [8, 4096, 128] f32. Distributes one batch per NeuronCore.
    """
    X = np.ascontiguousarray(X, dtype=np.float32)
    B = X.shape[0]
    assert X.shape == (B, N, D), X.shape
    bw = np.asarray(bandwidth, dtype=np.float32).reshape(1)

    nc = _get_nc()
    in_maps = [{"X": X[b], "bandwidth": bw} for b in range(B)]
    try:
        res = run_bass_kernel_spmd(nc, in_maps, core_ids=list(range(B)), **run_kwargs)
    except Exception:
        # The first execution after other jax-on-neuron work occasionally hits
        # a transient NRT_EXEC_UNIT_UNRECOVERABLE; a retry succeeds.
        res = run_bass_kernel_spmd(nc, in_maps, core_ids=list(range(B)), **run_kwargs)
    out = np.stack([res.results[b]["Y"] for b in range(B)], axis=0)
    kernel.last_results = res
    return out


if __name__ == "__main__":
    rng = np.random.default_rng(0)
    X = rng.standard_normal((8, N, D), dtype=np.float32)
    X /= np.linalg.norm(X, axis=-1, keepdims=True)
    out = kernel(X=X, bandwidth=np.float32(0.1))
    print("out shape", out.shape, "finite", np.isfinite(out).all())


# revision 30
# speedup vs baseline: 1.0013x; 1.0013x over previous
"""Trainium2 Bass kernel for GBMS mean-shift step (nn_GBMS_RNN_137438953906).

Math (per batch b):
    W = exp((X X^T - 1) / bandwidth^2)          [N, N]
    Y = (W @ X) / rowsum(W)                     [N, D]
    out = Y / max(||Y||_2, 1e-12)  (L2 norm along D)

rowsum(W) is a positive per-row scalar, so it cancels in the final L2
normalization; we never compute row sums.  Uniform scales on X cancel the
same way, so X is carried as 8*X (fp8-friendly range, exact power of 2).

Sharding: data-parallel over batch B=8 across the 8 NeuronCores.

Per-core dataflow (N=4096 as 8 column stripes of 512; W tiles are
[128 j-rows x 512 stripe-cols], 32 j-blocks per stripe):
  xt8[d64, 2, n] = 8*X^T in fp8e4m3  (PE half-transposes of bf16 8X + DVE
      convert; the [64,2] split is the DoubleRow matmul's paired-K layout)
  direct tile (jb, g):  S = xt8_jb^T xt8_g   (fp8 DoubleRow, 0.5 cyc/row)
                        W = exp(S/(64 b^2) - 1/b^2) -> bf16
                        (ACT, 2-tile batches, runtime scale/bias APs)
  symmetry reuse: W is symmetric, so the 4g tiles of stripe g above the
      diagonal are never recomputed: when stripe g' finishes the 4-tile
      group destined for stripe gd, ONE wide XBAR DMA-transpose turns the
      group [128, 2048] into wr [128, 16, 128] whose strided views
      wr[:, q::4, :] are ready-to-use transposed rhs tiles for stripe gd.
      This removes 44% of the exp work (ACT is the co-bottleneck with PE)
      and 44% of the S matmuls, at zero PE/ACT cost (DMA+HWDGE are idle).
  Y accumulation: yt[d, n512] += x16_jb^T @ W_tile  (bf16 matmuls, PSUM).
  Tail per stripe: yt -> bf16 stage -> PE transpose -> y16[n, d]; squares
      + row-reduce on DVE; fast-inverse-sqrt normalization (DVE bit trick
      + 1 Newton step, ~0.2% worst case); f32 stores, spread across the
      back stripes so the DVE/store burst overlaps PE work.

The emission schedule software-pipelines across stripes: each stripe's
S/exp stream runs one batch ahead of its Y-matmul stream, and a stripe's
leftover Y matmuls drain a few per cycle inside the next stripe so ACT
never waits behind a matmul burst at stripe boundaries.

fp8/bf16 error budget: measured 2.5e-3 overall at b=0.1 (output-side bf16
rounding) and 1.25e-2 at b=1.0 (fp8 X quantization common-mode term),
both under the 2e-2 gate.
"""

import sys

if "/opt/trn_rl_repo" not in sys.path:
    sys.path.insert(0, "/opt/trn_rl_repo")

import numpy as np

import concourse.mybir as mybir
from concourse import bacc
from concourse.tile import TileContext
from concourse.bass_utils import run_bass_kernel_spmd
from concourse.masks import make_identity

P = 128
N = 4096
D = 128
NB = N // P  # 32 row blocks
G = N // 512  # 8 column stripes
NCHUNK = 8  # input DMA chunks (4 row-blocks each)

F32 = mybir.dt.float32
BF16 = mybir.dt.bfloat16
FP8 = mybir.dt.float8e4
I32 = mybir.dt.int32
DR = mybir.MatmulPerfMode.DoubleRow

_CACHED_NC = None


def _build():
    nc = bacc.Bacc("TRN2", target_bir_lowering=False, debug=False)

    x_in = nc.dram_tensor("X", [N, D], F32, kind="ExternalInput")
    bw_in = nc.dram_tensor("bandwidth", [1], F32, kind="ExternalInput")
    y_out = nc.dram_tensor("Y", [N, D], F32, kind="ExternalOutput")

    x_src = x_in.rearrange("(jb p) d -> p jb d", p=P)  # [128, 32, 128] view
    y_dst = y_out.rearrange("(nb p) d -> p nb d", p=P)

    with TileContext(nc) as tc:
        with (
            tc.tile_pool(name="const", bufs=1) as const,
            tc.tile_pool(name="bigf32", bufs=1) as bigf32,
            tc.tile_pool(name="svpool", bufs=2) as sv_pool,
            tc.tile_pool(name="wrpool", bufs=17) as wr_pool,
            tc.tile_pool(name="sqpool", bufs=2) as sq_pool,
            tc.tile_pool(name="stgpool", bufs=2) as stg_pool,
            tc.tile_pool(name="spsum", bufs=2, space="PSUM") as s_pool,
            tc.tile_pool(name="ytpsum", bufs=2, space="PSUM") as yt_pool,
            tc.tile_pool(name="tppsum", bufs=2, space="PSUM") as tp_pool,
        ):
            # ---- input DMAs: chunk 0 first (it gates the pipeline) ----
            x_nat = bigf32.tile([P, NB, D], F32, tag="big", name="x_nat")
            cb = NB // NCHUNK  # 4 row blocks per chunk
            nc.sync.dma_start(x_nat[:, 0:cb, :], x_src[:, 0:cb, :])

            # bf16 identity first on the Pool queue so the PE warm-up can
            # start before the bandwidth SWDGE transfer completes
            identb = const.tile([P, P], BF16)
            make_identity(nc, identb[:])

            bw = const.tile([P, 1], F32)
            nc.gpsimd.dma_start(bw[:], bw_in[None, :].to_broadcast([P, 1]))

            for c in range(1, NCHUNK):
                nc.sync.dma_start(
                    x_nat[:, c * cb : (c + 1) * cb, :],
                    x_src[:, c * cb : (c + 1) * cb, :],
                )

            # ---- runtime scalars ----
            scr = const.tile([P, 5], F32)
            bsq = scr[:, 0:1]
            rb2 = scr[:, 1:2]
            negc = scr[:, 2:3]
            sc64 = scr[:, 3:4]
            dummy = scr[:, 4:5]
            # preload the Exp ACT table immediately (memset scratch input so
            # the 1.3us table load never waits for the bandwidth transfer)
            nc.vector.memset(dummy, 1.0)
            nc.scalar.activation(dummy, dummy, mybir.ActivationFunctionType.Exp)
            nc.vector.tensor_tensor(bsq, bw[:], bw[:], mybir.AluOpType.mult)
            nc.vector.reciprocal(rb2, bsq)  # 1/b^2
            nc.vector.tensor_scalar_mul(negc, rb2, -1.0)  # -1/b^2
            nc.vector.tensor_scalar_mul(sc64, rb2, 1.0 / 64.0)  # 1/(64 b^2)

            x16 = const.tile([P, NB, D], BF16)  # 8*X, Y-matmul lhsT
            xt8 = const.tile([64, 2, N], FP8)  # 8*X^T, S-matmul operands

            # PE warm-up junk transposes (ramp the PE clock during DMA wait)
            warm = s_pool.tile([P, 2, 512], F32, tag="s", name="warm")
            warmb = warm.bitcast(BF16)
            for t in range(4):
                nc.tensor.transpose(
                    warmb[:, t // 3, (t % 3) * P : (t % 3 + 1) * P],
                    identb[:],
                    identb[:],
                )

            chunks_done = [0]

            def emit_chunk(c):
                blk = slice(c * cb, (c + 1) * cb)
                nc.vector.tensor_scalar_mul(x16[:, blk, :], x_nat[:, blk, :], 8.0)
                xtp = tp_pool.tile([64, 2, 512], BF16, tag="tp", name="xtp")
                for o in range(cb):
                    jb = c * cb + o
                    for i in range(2):
                        nc.tensor.transpose(
                            xtp[:, i, o * P : (o + 1) * P],
                            x16[:, jb, i * 64 : (i + 1) * 64],
                            identb[:],
                        )
                nc.vector.tensor_copy(xt8[:, :, c * 512 : (c + 1) * 512], xtp[:])

            def need_chunks(upto):
                while chunks_done[0] <= min(upto, NCHUNK - 1):
                    emit_chunk(chunks_done[0])
                    chunks_done[0] += 1

            # ---- output staging ----
            y16 = const.tile([P, NB, D], BF16)  # [n_in_block, nb, d]
            ss_all = const.tile([P, NB], F32)
            tmp = const.tile([P, NB], F32)
            rcp = const.tile([P, NB], F32)
            magic = const.tile([P, NB], I32)
            shreg = const.tile([P, NB], I32)
            nc.vector.memset(magic[:], 0x5F3759DF)
            y_stage_box = [None]

            def normalize_and_store(g0, g1):
                """L2-normalize output rows of stripes [g0, g1) and DMA out.
                1/norm via the fast-inverse-sqrt bit trick + ONE Newton step
                (max rel err ~0.2%, well inside the error budget).  ss is
                always well away from 0: W's diagonal is ~1 so |Y| >= ~8."""
                y_stage = y_stage_box[0]
                lo, hi = g0 * 4, g1 * 4  # nb range
                ss = ss_all[:, lo:hi]
                rs = rcp[:, lo:hi]
                tm = tmp[:, lo:hi]
                nc.vector.tensor_scalar(
                    shreg[:, lo:hi],
                    ss.bitcast(I32),
                    1,
                    None,
                    mybir.AluOpType.logical_shift_right,
                )
                nc.vector.tensor_tensor(
                    rs.bitcast(I32),
                    magic[:, lo:hi],
                    shreg[:, lo:hi],
                    mybir.AluOpType.subtract,
                )
                nc.vector.tensor_tensor(tm, rs, rs, mybir.AluOpType.mult)
                nc.vector.tensor_tensor(tm, tm, ss, mybir.AluOpType.mult)
                nc.vector.tensor_scalar(
                    tm, tm, -0.5, 1.5, mybir.AluOpType.mult, mybir.AluOpType.add
                )
                nc.vector.tensor_tensor(rs, rs, tm, mybir.AluOpType.mult)
                for nb in range(lo, hi):
                    nc.vector.tensor_scalar_mul(
                        y_stage[:, nb, :], y16[:, nb, :], rcp[:, nb : nb + 1]
                    )
                mid = (lo + hi) // 2
                nc.sync.dma_start(y_dst[:, lo:mid, :], y_stage[:, lo:mid, :])
                nc.sync.dma_start(y_dst[:, mid:hi, :], y_stage[:, mid:hi, :])

            def make_tail(g, stg, fine=False):
                """Tail of stripe g: stg (= yt in bf16) -> y16[n, d] via one
                wide XBAR DMA-transpose (PE-free), then per-block fused
                square+reduce (DVE tensor_tensor_reduce).  fine mode (last
                stripe) uses PE transposes instead: lower latency since the
                DMA path costs ~1.7us end-to-end."""

                def tail():
                    nbs0 = g * 4
                    if fine:
                        for h in range(2):
                            tp = tp_pool.tile([P, 4, P], BF16, tag="tp", name="tp")
                            for t in range(2):
                                tt = h * 2 + t
                                nc.tensor.transpose(
                                    tp[:, t, :],
                                    stg[:, tt * P : (tt + 1) * P],
                                    identb[:],
                                )
                            nc.vector.tensor_copy(
                                y16[:, nbs0 + h * 2 : nbs0 + h * 2 + 2, :],
                                tp[:, 0:2, :],
                            )
                            sqt = sq_pool.tile([P, 2, P], F32, tag="sq", name="sqt")
                            nbs = slice(nbs0 + h * 2, nbs0 + h * 2 + 2)
                            nc.vector.tensor_tensor(
                                sqt[:], y16[:, nbs, :], y16[:, nbs, :],
                                mybir.AluOpType.mult,
                            )
                            nc.vector.tensor_reduce(
                                ss_all[:, nbs],
                                sqt[:],
                                axis=mybir.AxisListType.X,
                                op=mybir.AluOpType.add,
                            )
                    else:
                        nc.sync.dma_start_transpose(
                            y16[:, nbs0 : nbs0 + 4, :], stg[:]
                        )
                        sqt = sq_pool.tile([P, 4, P], F32, tag="sq", name="sqt")
                        nbs = slice(nbs0, nbs0 + 4)
                        nc.gpsimd.tensor_tensor(
                            sqt[:], y16[:, nbs, :], y16[:, nbs, :],
                            mybir.AluOpType.mult,
                        )
                        nc.vector.tensor_reduce(
                            ss_all[:, nbs],
                            sqt[:],
                            axis=mybir.AxisListType.X,
                            op=mybir.AluOpType.add,
                        )

                return tail

            wr_tiles = {}  # (gs, gd) -> wide-transposed 4-tile group
            # cross-stripe pipeline state: leftover Y matmuls of the previous
            # stripe drain a few per cycle so the next stripe's S/exp are
            # never stuck behind a matmul burst; then its stg copy + tail.
            state = {"carry": [], "stg": None, "tail": None}
            CR = 12  # carry drain rate per cycle

            # per-stripe Y-accumulation state persists across the loop so a
            # stripe's reuse-Y matmuls can be pre-released during the
            # previous (ACT-bound) stripe
            sstates = [{"yt": None, "n": 0, "pre": 0} for _ in range(G)]

            def emit_y_for(gy, jb, rhs):
                st = sstates[gy]
                if st["yt"] is None:
                    st["yt"] = yt_pool.tile([P, 512], F32, tag="yt", name="yt")
                nc.tensor.matmul(
                    st["yt"][:],
                    x16[:, jb, :],
                    rhs,
                    start=(st["n"] == 0),
                    stop=(st["n"] == 31),
                )
                st["n"] += 1

            # ---- main loop over column stripes ----
            for g in range(G):
                ndirect = 32 - 4 * g
                batches = [[s, s + 1] for s in range(0, ndirect, 2)]

                sv = sv_pool.tile([P, 32, 512], BF16, tag="sv", name="sv")

                def emit_y(jb, rhs, g=g):
                    emit_y_for(g, jb, rhs)

                rq = []
                for jb in range(sstates[g]["pre"], 4 * g):
                    gs, q = jb // 4, jb % 4
                    rq.append((jb, wr_tiles[(gs, g)][:, q:16:4, :]))
                rpc = -(-len(rq) // len(batches))  # ceil: spread over cycles

                dq_ready = []  # direct (jb, rhs) whose exp has been emitted

                def emit_batch(slots):
                    # S matmuls (fp8 DoubleRow) + exp batch -> sv (bf16)
                    if g == 0:
                        need_chunks(min(slots[-1] // cb + 1, NCHUNK - 1))
                    s_t = s_pool.tile([P, 2, 512], F32, tag="s", name="s_t")
                    for q, sl in enumerate(slots):
                        jb = 4 * g + sl
                        nc.tensor.matmul(
                            s_t[:, q, :],
                            xt8[:, :, jb * P : (jb + 1) * P],
                            xt8[:, :, g * 512 : (g + 1) * 512],
                            start=True,
                            stop=True,
                            perf_mode=DR,
                        )
                    nc.scalar.activation(
                        sv[:, slots[0] : slots[-1] + 1, :],
                        s_t[:],
                        mybir.ActivationFunctionType.Exp,
                        bias=negc,
                        scale=sc64,
                    )
                    for sl in slots:
                        dq_ready.append((4 * g + sl, sv[:, sl, :]))
                        if sl % 4 == 3 and sl >= 4:
                            gd = g + sl // 4
                            wr = wr_pool.tile(
                                [P, 16, P], BF16, tag="wr", name="wr"
                            )
                            wr_tiles[(g, gd)] = wr
                            nc.sync.dma_start_transpose(
                                wr[:],
                                sv[:, sl - 3 : sl + 1, :].rearrange(
                                    "p a b -> p (a b)"
                                ),
                            )

                for k in range(len(batches)):
                    # S/exp run one batch ahead of the Y stream
                    if k == 0:
                        emit_batch(batches[0])
                        if len(batches) > 1:
                            emit_batch(batches[1])
                    elif k + 1 < len(batches):
                        emit_batch(batches[k + 1])
                    slots = batches[k]
                    # drain the previous stripe's leftovers, then its stg
                    # copy + tail
                    for _ in range(CR):
                        if state["carry"]:
                            state["carry"].pop(0)()
                    if not state["carry"] and state["stg"] is not None:
                        state["stg"]()
                        state["stg"] = None
                        state["tail"]()
                        state["tail"] = None
                    # reuse-Y fillers (no ACT dependency)
                    for _ in range(rpc):
                        if rq:
                            emit_y(*rq.pop(0))
                    # direct-Y, trailing the lookahead exp batches deeply
                    # (extra slack hides the exp->matmul semaphore joins)
                    while len(dq_ready) > 3 * len(slots):
                        emit_y(*dq_ready.pop(0))
                    # normalization spread over the back stripes
                    if g == G - 2 and k == 3:
                        y_stage_box[0] = bigf32.tile(
                            [P, NB, D], F32, tag="big", name="y_stage"
                        )
                        normalize_and_store(0, 3)
                    if g == G - 1 and k == 0:
                        normalize_and_store(3, 5)
                    if g == G - 1 and k == 1:
                        normalize_and_store(5, G - 1)

                def make_carry(e, emit_y=emit_y):
                    return lambda: emit_y(*e)

                state["carry"] = [make_carry(e) for e in rq + dq_ready]

                def make_stg(g=g):
                    def stg_fn():
                        assert sstates[g]["n"] == 32, (g, sstates[g]["n"])
                        stg = stg_pool.tile(
                            [P, 512], BF16, tag="stg", name="stg"
                        )
                        nc.vector.tensor_copy(stg[:], sstates[g]["yt"][:])
                        state["tail"] = make_tail(g, stg, fine=True)

                    return stg_fn

                state["stg"] = make_stg()

            while state["carry"]:
                state["carry"].pop(0)()
            state["stg"]()
            state["tail"]()
            normalize_and_store(G - 1, G)

    nc.compile()
    return nc


def _get_nc():
    global _CACHED_NC
    if _CACHED_NC is None:
        _CACHED_NC = _build()
    return _CACHED_NC


def kernel(X: np.ndarray, bandwidth: np.ndarray, **run_kwargs):
    """Full-input entry point: X [8, 4096, 128] f32, bandwidth scalar f32.

    Returns [8, 4096, 128] f32. Distributes one batch per NeuronCore.
    """
    X = np.ascontiguousarray(X, dtype=np.float32)
    B = X.shape[0]
    assert X.shape == (B, N, D), X.shape
    bw = np.asarray(bandwidth, dtype=np.float32).reshape(1)

    nc = _get_nc()
    in_maps = [{"X": X[b], "bandwidth": bw} for b in range(B)]
    try:
        res = run_bass_kernel_spmd(nc, in_maps, core_ids=list(range(B)), **run_kwargs)
    except Exception:
        # The first execution after other jax-on-neuron work occasionally hits
        # a transient NRT_EXEC_UNIT_UNRECOVERABLE; a retry succeeds.
        res = run_bass_kernel_spmd(nc, in_maps, core_ids=list(range(B)), **run_kwargs)
    out = np.stack([res.results[b]["Y"] for b in range(B)], axis=0)
    kernel.last_results = res
    return out


if __name__ == "__main__":
    rng = np.random.default_rng(0)
    X = rng.standard_normal((8, N, D), dtype=np.float32)
    X /= np.linalg.norm(X, axis=-1, keepdims=True)
    out = kernel(X=X, bandwidth=np.float32(0.1))
    print("out shape", out.shape, "finite", np.isfinite(out).all())
